# revision 6
# baseline (speedup 1.0000x reference)
"""DFFN Trainium2 kernel v2 for nn_DFFN_81535659147929.

Pipeline: project_in (1x1 conv, 64->340) -> per-8x8-patch rFFT2 * learned
filter -> irFFT2 -> depthwise 3x3 conv -> GELU gate -> project_out (170->64).

v2 redesign vs baseline:
  - Stage A per 2-patch block: Fx = DFT(x) first (K=128 pix, out [64ch,128f]),
    then project_in in freq space (K=64, FD=352, rhs=w_inT const), then
    lambda multiply, then inverse per channel chunk.  Cheaper evacs.
  - z stays in SBUF in a 40-row ring (3 chunks x [128, 40*256] bf16);
    no DRAM bounce.
  - dwconv: 16-row windows (14 valid out rows), 8ch x 16row partition
    tiles, 3 col-shifted banded matmuls; gathers batched as ONE DMA per
    (chunk, window) [2MB] instead of per-subgroup.
  - gate (gelu(x1)*x2) right out of psum; restructure to [gate-ch, pix]
    via one SWDGE DMA per (pair, window); project_out per window from
    SBUF; output bf16, cast to f32 on host.

Sharding: core = (image b = core//2, row half core%2), 8-row patch-aligned
halo; weights replicated.
"""

import numpy as np
import ml_dtypes

import concourse.bass as bass
import concourse.mybir as mybir
from concourse import bacc, tile
from concourse.bass_utils import run_bass_kernel_spmd

BF16 = mybir.dt.bfloat16
F32 = mybir.dt.float32

DIM = 64
C2 = 340
P = 8
B, H, W = 4, 256, 256
N_CORES = 8
ROWS = H // 2
HALO = P
NCOL = 352          # 22 groups x 16, incl 12 pad channels
NGRP = 22
NPAIR = 11          # gate pairs (x1 group 2m, x2 group 2m+1)
RH = ROWS + 2 * HALO            # 144
NPR = RH // P                   # 18 patch rows
NBLK = NPR * 16                 # 288 blocks (2 patches each)
RING = 48                       # z ring rows
NWIN = 10                       # dwconv windows (14 valid rows each)
WSTEP = 14

_cache = {}


# ----------------------------------------------------------------- host math

def _build_basis():
    rows = []
    seen = set()
    p1, p2 = np.meshgrid(np.arange(P), np.arange(P), indexing="ij")
    for u in range(P):
        for v in range(P):
            if (u, v) in seen:
                continue
            nu, nv = (-u) % P, (-v) % P
            th = 2 * np.pi * (u * p1 + v * p2) / P
            if (nu, nv) == (u, v):
                rows.append((np.cos(th) / 8.0).ravel())
            else:
                seen.add((nu, nv))
                rows.append((np.sqrt(2) / 8.0) * np.cos(th).ravel())
                rows.append((np.sqrt(2) / 8.0) * np.sin(th).ravel())
            seen.add((u, v))
    return np.array(rows, dtype=np.float64)


def _lam_for(fft_w, C):
    basis = C.reshape(64, P, P)
    F = np.fft.rfft2(basis)
    w = fft_w.reshape(C2, 1, P, P // 2 + 1).astype(np.float64)
    r = np.fft.irfft2(F[None] * w, s=(P, P))
    return np.einsum('kpq,ckpq->ck', basis, r)      # [C2, 64]


def _col_to_c2():
    cols = np.full(NCOL, -1, np.int64)
    for g in range(NGRP):
        m, half = divmod(g, 2)
        for j in range(16):
            ch = 16 * m + j
            if ch < 170:
                cols[g * 16 + j] = ch + 170 * half
    return cols


def _pix_maps():
    C = _build_basis()
    CCrhs = np.zeros((128, 128))
    for pc2 in range(2):
        for f in range(64):
            for p1 in range(P):
                for p2 in range(P):
                    k = p1 * 16 + pc2 * 8 + p2
                    CCrhs[pc2 * 64 + f, k] = C[f, p1 * 8 + p2]
    return CCrhs.T.copy(), CCrhs, C


def _prep_weights(w_in, w_dw, fft_w, w_out):
    CClhsT, CCrhs, C = _pix_maps()
    lam = _lam_for(fft_w, C)
    cols = _col_to_c2()
    valid = cols >= 0

    w_inT = np.zeros((64, NCOL))
    w_inT[:, valid] = w_in.T[:, cols[valid]]
    w_in2 = np.concatenate([w_inT, w_inT], axis=0)          # [128, NCOL]

    lam_t = np.zeros((128, NCOL))
    lam_sel = np.zeros((NCOL, 64))
    lam_sel[valid] = lam[cols[valid]]
    lam_t[:] = np.tile(lam_sel.T, (2, 1))[:128]

    dw = w_dw.reshape(C2, 3, 3)
    # dwconv banded lhsT per (subgroup of 8 ch, dx): [128, 128]
    # in partition (c,i) = c*16+i, out partition (t,c) = t*8+c (t-major so
    # the valid rows t<14 are a contiguous partition prefix),
    # T[(c,i),(t,c)] = dw[ch, i-t, dx] for i-t in {0,1,2}, t < 14
    dd = np.zeros((128, 44 * 3 * 128))
    for sg in range(44):
        for dx in range(3):
            blk = np.zeros((128, 128))
            for c in range(8):
                c2 = cols[sg * 8 + c]
                if c2 < 0:
                    continue
                for t in range(WSTEP):
                    for dy in range(3):
                        i = t + dy
                        if i < 16:
                            blk[c * 16 + i, t * 8 + c] = dw[c2, dy, dx]
            dd[:, (sg * 3 + dx) * 128:(sg * 3 + dx + 1) * 128] = blk

    # project_out lhsT chunks: gate channel gch = 16m+8s+c
    wo0 = np.zeros((128, 64))
    wo0[0:128] = w_out.T[0:128]
    wo1 = np.zeros((48, 64))
    wo1[0:42] = w_out.T[128:170]
    bf = ml_dtypes.bfloat16
    return {
        "w_in2": w_in2.astype(bf),
        "cclhsT": CClhsT.astype(bf),
        "ccrhs": CCrhs.astype(bf),
        "lam_t": lam_t.astype(bf),
        "dd": dd.astype(bf),
        "wo0": wo0.astype(bf),
        "wo1": wo1.astype(bf),
    }


# ---------------------------------------------------------------- bass build

def build_nc(dbg=False):
    G = mybir.ActivationFunctionType.Gelu
    chunks = [(0, 128), (128, 128), (256, 96)]
    # gather-ready patch row (emission point) per window, and deadline checks
    ready = {4: [0], 6: [1], 8: [2], 9: [3], 11: [4], 13: [5],
             15: [6], 16: [7], 17: [8, 9]}

    nc = bacc.Bacc("TRN2", target_bir_lowering=False, debug=False,
                   num_devices=N_CORES)
    x_d = nc.dram_tensor("x", [128, NPR * 16 * 64], BF16, kind="ExternalInput")
    win2_d = nc.dram_tensor("w_in2", [128, NCOL], BF16, kind="ExternalInput")
    cclhsT_d = nc.dram_tensor("cclhsT", [128, 128], BF16, kind="ExternalInput")
    ccrhs_d = nc.dram_tensor("ccrhs", [128, 128], BF16, kind="ExternalInput")
    lam_d = nc.dram_tensor("lam_t", [128, NCOL], BF16, kind="ExternalInput")
    dd_d = nc.dram_tensor("dd", [128, 44 * 3 * 128], BF16,
                          kind="ExternalInput")
    wo0_d = nc.dram_tensor("wo0", [128, 64], BF16, kind="ExternalInput")
    wo1_d = nc.dram_tensor("wo1", [48, 64], BF16, kind="ExternalInput")
    out_d = nc.dram_tensor("out", [64, ROWS * W], BF16, kind="ExternalOutput")
    # z bounce: row index = (ci, c_sub, g), cols = (row, w)
    zkind = "ExternalOutput" if dbg else "Internal"
    zd = nc.dram_tensor("zd", [3 * 8 * 16, RH * W], BF16, kind=zkind)
    gd = nc.dram_tensor("gd", [128, NWIN * WSTEP * W], BF16, kind=zkind)
    gtd = nc.dram_tensor("gtd", [128, 512], BF16, kind=zkind)
    gbuf = nc.dram_tensor("gbuf", [176, ROWS * W], BF16, kind="Internal")

    with tile.TileContext(nc) as tc:
        with tc.tile_pool(name="consts", bufs=1) as cpool:
            w_in2 = cpool.tile([128, NCOL], BF16)
            nc.sync.dma_start(out=w_in2[:], in_=win2_d[:])
            cclhsT = cpool.tile([128, 128], BF16)
            nc.sync.dma_start(out=cclhsT[:], in_=cclhsT_d[:])
            ccrhs = cpool.tile([128, 128], BF16)
            nc.sync.dma_start(out=ccrhs[:], in_=ccrhs_d[:])
            lam_t = cpool.tile([128, NCOL], BF16)
            nc.sync.dma_start(out=lam_t[:], in_=lam_d[:])
            dd = cpool.tile([128, 44 * 3 * 128], BF16)
            nc.sync.dma_start(out=dd[:], in_=dd_d[:])
            wo0 = cpool.tile([128, 64], BF16)
            nc.sync.dma_start(out=wo0[:], in_=wo0_d[:])
            wo1 = cpool.tile([48, 64], BF16)
            nc.sync.dma_start(out=wo1[:], in_=wo1_d[:])
            zd_v = zd[:].rearrange("(ci c g) (r w) -> ci c g r w",
                                   ci=3, c=8, g=16, r=RH)

            with (
                tc.tile_pool(name="xs", bufs=6) as xpool,
                tc.tile_pool(name="zs", bufs=2) as zspool,
                tc.tile_pool(name="sf", bufs=6) as sfpool,
                tc.tile_pool(name="sb", bufs=8) as sbpool,
                tc.tile_pool(name="zt", bufs=3) as ztpool,
                tc.tile_pool(name="ge", bufs=6) as gepool,
                tc.tile_pool(name="gt", bufs=6) as gtpool,
                tc.tile_pool(name="gp", bufs=2) as gppool,
                tc.tile_pool(name="ob", bufs=1) as obpool,
                tc.tile_pool(name="fb", bufs=2, space="PSUM") as fbpool,
                tc.tile_pool(name="pz", bufs=2, space="PSUM") as pzpool,
                tc.tile_pool(name="pq", bufs=4, space="PSUM") as pqpool,
            ):
                # x loads, per patch-row: [128, 16 blocks * 64 ch]
                xts = {}

                def load_x(p):
                    xt = xpool.tile([128, 16 * 64], BF16, tag="x")
                    nc.sync.dma_start(out=xt[:],
                                      in_=x_d[:, p * 1024:(p + 1) * 1024])
                    xts[p] = xt

                for p in range(4):
                    load_x(p)

                # HAM warmup while DMAs land
                wps = fbpool.tile([128, NCOL], F32, tag="fb", name="warm")
                for _ in range(40):
                    nc.tensor.matmul(wps[:, 0:128], cclhsT[:], ccrhs[:],
                                     start=True, stop=True)

                def emit_a(p):
                    """Stage A for patch row p: 16 blocks -> z rows in DRAM."""
                    xt = xts.pop(p)
                    zs = zspool.tile([128, 3 * 8 * W], BF16, tag="zs")
                    zs_v = zs[:].rearrange("c (ci r w) -> c ci r w",
                                           ci=3, r=8)
                    for pq in range(4):
                        sBs = []
                        for k in range(2):          # pairs in group
                            kp = pq * 2 + k
                            psF = fbpool.tile([128, 128], F32, tag="fb",
                                              name="psF")
                            nc.tensor.matmul(psF[:],
                                             xt[:, kp * 128:(kp + 1) * 128],
                                             cclhsT[:], start=True, stop=True)
                            sF = sfpool.tile([128, 128], BF16, tag="sF")
                            nc.scalar.copy(sF[:], psF[:])
                            for b in range(2):
                                psB = fbpool.tile([128, NCOL], F32, tag="fb",
                                                  name="psB")
                                nc.tensor.matmul(psB[:],
                                                 sF[b * 64:(b + 1) * 64, :],
                                                 w_in2[b * 64:(b + 1) * 64, :],
                                                 start=True, stop=True)
                                sB = sbpool.tile([128, NCOL], BF16, tag="sB")
                                nc.vector.tensor_mul(sB[:], psB[:], lam_t[:])
                                sBs.append(sB)
                        for ci, (c0, m) in enumerate(chunks):
                            psZ = pzpool.tile([128, 512], F32, tag="pz")
                            for pj in range(4):
                                nc.tensor.matmul(
                                    psZ[0:m, pj * 128:(pj + 1) * 128],
                                    sBs[pj][:, c0:c0 + m], ccrhs[:],
                                    start=(pj == 0), stop=(pj == 3))
                            src = psZ[0:m, :].rearrange(
                                "c (pj p1 q) -> c p1 pj q", pj=4, p1=8)
                            dst = zs_v[0:m, ci, :,
                                       pq * 64:(pq + 1) * 64].rearrange(
                                "c r (pj q) -> c r pj q", pj=4)
                            if ci == 2:
                                nc.vector.tensor_copy(dst, src)
                            else:
                                nc.scalar.copy(dst, src)
                    # write z rows to DRAM: one DMA per chunk
                    # zd row layout: (c_sub, ci, g)
                    zdr = zd[:].rearrange("(c ci g) x -> ci g c x",
                                          ci=3, c=8, g=16)
                    for ci, (c0, m) in enumerate(chunks):
                        ng = m // 8
                        nc.sync.dma_start(
                            out=zdr[ci, 0:ng, :,
                                    p * 8 * W:(p + 1) * 8 * W],
                            in_=zs[0:m, ci * 8 * W:(ci + 1) * 8 * W])

                def emit_gather(w):
                    """Prefetch z window w from DRAM into one zt tile.

                    zt cols = (ci*16+g, w); one DMA per (chunk, c_sub).
                    """
                    zr0 = WSTEP * w + 7                  # first z row needed
                    nrow = min(16, RH - zr0)
                    zdg = zd[:].rearrange("(c ci g) (r w) -> ci c r g w",
                                          c=8, ci=3, g=16, r=RH)
                    zt = ztpool.tile([128, 48 * W], BF16, tag="zt")
                    ztv = zt[:].rearrange("p (ci g w) -> ci p g w",
                                          ci=3, g=16)
                    for ci, (c0, m) in enumerate(chunks):
                        ng = m // 8
                        for cs in range(8):
                            dst = ztv[ci, cs * 16:cs * 16 + nrow, 0:ng, :]
                            eng = (nc.sync, nc.gpsimd, nc.scalar)[cs % 3]
                            eng.dma_start(
                                out=dst,
                                in_=zdg[ci, cs, zr0:zr0 + nrow, 0:ng, :])
                    return zt

                def emit_proj(w, gps):
                    """project_out for window w from restructured gate chs."""
                    gp0, gp1 = gps
                    nv = min(WSTEP, ROWS - WSTEP * w)
                    ncols = nv * W
                    ob = obpool.tile([64, WSTEP * W], BF16, tag="ob")
                    nct = (ncols + 511) // 512
                    for ct in range(nct):
                        c0_, c1 = ct * 512, min((ct + 1) * 512, ncols)
                        po = pqpool.tile([64, 512], F32, tag="q", name="po")
                        nc.tensor.matmul(po[:, 0:c1 - c0_], wo0[:],
                                         gp0[:, c0_:c1], start=True,
                                         stop=False)
                        nc.tensor.matmul(po[:, 0:c1 - c0_], wo1[:],
                                         gp1[0:48, c0_:c1], start=False,
                                         stop=True)
                        nc.vector.tensor_copy(ob[:, c0_:c1],
                                              po[:, 0:c1 - c0_])
                    nc.sync.dma_start(
                        out=out_d[:, WSTEP * w * W:WSTEP * w * W + ncols],
                        in_=ob[:, 0:ncols])

                def emit_b(w, zt):
                    """Stage B for window w: dwconv + gate; returns gp tiles."""
                    nv = min(WSTEP, ROWS - WSTEP * w)    # valid out rows
                    gp0 = gppool.tile([128, WSTEP * W], BF16, tag="gp0")
                    gp1 = gppool.tile([48, WSTEP * W], BF16, tag="gp1")
                    dxs = [(1, 0, 0, 256), (0, 0, 1, 255), (2, 1, 0, 255)]
                    for m_ in range(NPAIR):
                        ci = m_ // 4
                        goff = (m_ % 4) * 4             # subgroup offset
                        pss = []
                        for half in range(2):
                            ps = pqpool.tile([128, 512], F32, tag="q",
                                             name=f"q{half}")
                            for s in range(2):
                                sg = 16 * ci + goff + 2 * half + s
                                gl = ci * 16 + goff + 2 * half + s
                                for dx, wi0, wo0_, wn in dxs:
                                    lhs = dd[:, (sg * 3 + dx) * 128:
                                             (sg * 3 + dx + 1) * 128]
                                    rhs = zt[:, gl * 256 + wi0:
                                             gl * 256 + wi0 + wn]
                                    nc.tensor.matmul(
                                        ps[:, s * 256 + wo0_:
                                           s * 256 + wo0_ + wn],
                                        lhs, rhs,
                                        start=(dx == 1), stop=(dx == 2))
                            pss.append(ps)
                        ge = gepool.tile([128, 512], BF16, tag="ge")
                        nc.scalar.activation(ge[:], pss[0][:], G)
                        gt = gtpool.tile([128, 512], BF16, tag="gt")
                        nc.vector.tensor_mul(gt[:], ge[:], pss[1][:])
                        if dbg and w == 1 and m_ == 0:
                            nc.sync.dma_start(out=gtd[:], in_=gt[:])
                        # restructure: (c,t) partitions -> gate-ch partitions
                        gbv = gbuf[:].rearrange("gc (r w) -> r gc w", w=W)
                        for s_ in range(2):
                            src = gt[0:nv * 8, s_ * 256:(s_ + 1) * 256]
                            gc0 = 16 * m_ + 8 * s_
                            dst = gbv[WSTEP * w:WSTEP * w + nv,
                                      gc0:gc0 + 8, :]
                            eng = nc.gpsimd if s_ == 0 else nc.sync
                            eng.dma_start(out=dst, in_=src)
                    # load restructured gate channels back from DRAM
                    nc.gpsimd.dma_start(
                        out=gp0[:, 0:nv * W],
                        in_=gbuf[0:128, WSTEP * w * W:(WSTEP * w + nv) * W])
                    nc.sync.dma_start(
                        out=gp1[:, 0:nv * W],
                        in_=gbuf[128:176, WSTEP * w * W:(WSTEP * w + nv) * W])
                    if dbg:
                        nc.sync.dma_start(
                            out=gd[:, w * WSTEP * W:w * WSTEP * W + nv * W],
                            in_=gp0[:, 0:nv * W])
                    return gp0, gp1

                # software-pipelined schedule: gathers two steps ahead of
                # dwconv+gate, proj one step behind
                pend_g = {}          # w -> zt (gather aged 1 step)
                pend_g2 = {}         # w -> zt (fresh gather)
                pend_p = {}          # w -> gp tiles

                def step_b(p):
                    for w in list(pend_p):
                        emit_proj(w, pend_p.pop(w))
                    for w in list(pend_g):
                        pend_p[w] = emit_b(w, pend_g.pop(w))
                    pend_g.update(pend_g2)
                    pend_g2.clear()
                    for w in ready.get(p, []):
                        pend_g2[w] = emit_gather(w)

                for p in range(NPR):
                    if p + 4 < NPR:
                        load_x(p + 4)
                    emit_a(p)
                    step_b(p)
                for w in list(pend_g) + list(pend_g2):
                    zt = pend_g.pop(w, None) or pend_g2.pop(w)
                    pend_p[w] = emit_b(w, zt)
                for w in list(pend_p):
                    emit_proj(w, pend_p.pop(w))

    nc.compile()
    return nc


# ----------------------------------------------------------------- interface

def _get_program(dbg=False):
    key = ("nc", dbg)
    if key not in _cache:
        _cache[key] = build_nc(dbg)
    return _cache[key]


def _shard_x(x):
    """Per core: pixel-major blocks [128, (pr, blk16, ch64)]."""
    shards = []
    for c in range(N_CORES):
        b, hh = divmod(c, 2)
        r0 = hh * ROWS
        xs = np.zeros((DIM, RH, W), np.float32)
        lo, hi = r0 - HALO, r0 + ROWS + HALO
        slo, shi = max(lo, 0), min(hi, x.shape[2])
        xs[:, slo - lo:shi - lo] = x[b, :, slo:shi]
        # [c, pr, p1, pcp, pc2, p2] -> [(p1 pc2 p2), (pr pcp c)]
        xp = xs.reshape(DIM, NPR, P, 16, 2, P).transpose(2, 4, 5, 1, 3, 0)
        shards.append(np.ascontiguousarray(xp).reshape(128, NPR * 16 * 64)
                      .astype(ml_dtypes.bfloat16))
    return shards


def _run(x, w_in, w_dw, fft_w, w_out, trace=False, dbg=False):
    nc = _get_program(dbg)
    wts = _prep_weights(np.asarray(w_in, np.float32),
                        np.asarray(w_dw, np.float32).reshape(C2, 3, 3),
                        np.asarray(fft_w, np.float32),
                        np.asarray(w_out, np.float32))
    shards = _shard_x(np.asarray(x, np.float32))
    in_maps = [{"x": s, **wts} for s in shards]
    res = run_bass_kernel_spmd(nc, in_maps, core_ids=list(range(N_CORES)),
                               trace=trace)
    out = np.zeros((B, DIM, H, W), np.float32)
    for c in range(N_CORES):
        b, hh = divmod(c, 2)
        out[b, :, hh * ROWS:(hh + 1) * ROWS] = (
            res.results[c]["out"].astype(np.float32).reshape(DIM, ROWS, W))
    if dbg:
        return out, res.exec_time_ns, res.results
    return out, res.exec_time_ns


def kernel(x, w_in, w_dw, fft_w, w_out):
    out, _ = _run(x, w_in, w_dw, fft_w, w_out, trace=False)
    return out
